# revision 38
# baseline (speedup 1.0000x reference)
"""Trainium2 Bass kernel for CantorMultiheadFusion (sliding-window multi-head
attention, window K=64, H=16 heads, D=64, DIM=1024, x [2, 2048, 1024]).

Sharding: pure data-parallel over (batch, seq-quarter) -> 8 cores, 512 queries
each. Each core gets a 575-column key window of x^T whose out-of-range columns
are clamp-replicated on the host (exactly reproducing the reference's index
clamping), making the on-device program uniform across cores: plain banded
attention with a constant band mask, no collectives.

All matmuls run bf16 with K=128 contraction (per-head K tiles are zero-padded
to 128 partitions so the other head's Q rows are annihilated). The softmax
denominator comes from a ones-column appended to V.

v7 performance structure (from v5 trace analysis: Tensor 79.5% busy with a
5.7us PE hole at the Q->K phase boundary caused by wk DMAs completing at the
~330GB/s aggregate DMA cap + scalar-engine FIFO head-of-line blocking, ~4us
of startup DMA pacing, and a 5.5us output-drain tail):
  - DMA queues ordered by need-time: the scalar HW queue carries ONLY xw
    (drained early, so the Q/K PSUM-drain copies on the scalar engine are
    never stuck behind DMA ring-credit waits); the sync HW queue carries wq
    then wk then the small band/bias tensors; the gpsimd software queue runs
    the (trimmed) pad memsets first, which naturally delays the wv/wo issues
    until the early-needed tensors have used the bandwidth.
  - memsets only cover the regions that the K/V projection copies don't
    write (kt2 other-head rows + column pad, vt ones-column + tail rows).
  - the -30000 band bias stays a PE matmul: a GpSimd 0/1 mask multiply
    (663ns/half), DVE masks, and DMA-XBAR transposes (1.24us engine time
    each) were all measured slower than the PE doing it.
  - O projection issues both output-half accumulations for chunks 0-6
    before the last transpose, so the PE never waits on the chunk-7
    normalize; phase-filler matmuls dropped (the reordered DMAs keep the PE
    dense, HAM stays at 8/8 end to end).
  - output is written eh-major ([2, CH, 512]) so each (qblock, eh) store is
    one contiguous 128KB DMA on the idle gpsimd queue; the final store's
    bias-add is folded into PSUM by a K=1 ones-matmul so the last two
    drains + three stores all run in parallel on separate engines.
"""

import sys

for _p in ("/opt/trn_rl_repo", "/root/.axon_site/_ro/trn_rl_repo"):
    if _p not in sys.path:
        sys.path.append(_p)

import numpy as np
import ml_dtypes

import concourse.bass as bass
import concourse.tile as tile
from concourse import bacc, mybir
from concourse.bass_utils import run_bass_kernel_spmd
from concourse.masks import make_identity

BF16 = ml_dtypes.bfloat16

B, S, DIM = 2, 2048, 1024
H, D, K = 16, 64, 64
HALF = K // 2            # 32
CH = 512                 # queries per core
W = CH + K - 1           # 575 key-window columns per core
WPAD = CH + 128          # kt2 padded width (chunk-B reads for the last qblock)
NB = DIM // 128          # 8 dim chunks
N_CORES = 8
QB = CH // 128           # 4 query blocks per core
SCALE = 1.0 / np.sqrt(D)

_CACHED = {}


def _build_nc():
    fp32 = mybir.dt.float32
    bf16 = mybir.dt.bfloat16
    Exp = mybir.ActivationFunctionType.Exp

    nc = bacc.Bacc("TRN2", target_bir_lowering=False, debug=False,
                   num_devices=N_CORES)

    xw_d = nc.dram_tensor("xw", [DIM, W], bf16, kind="ExternalInput")
    wq_d = nc.dram_tensor("wq", [DIM, DIM], bf16, kind="ExternalInput")
    wk_d = nc.dram_tensor("wk", [DIM, DIM], bf16, kind="ExternalInput")
    wv_d = nc.dram_tensor("wv", [DIM, DIM], bf16, kind="ExternalInput")
    wo_d = nc.dram_tensor("wo", [DIM, DIM], bf16, kind="ExternalInput")
    bo_d = nc.dram_tensor("bo", [1, DIM], bf16, kind="ExternalInput")
    band_d = nc.dram_tensor("band", [128, 512], bf16, kind="ExternalInput")
    out_d = nc.dram_tensor("out", [2, CH, 512], bf16, kind="ExternalOutput")

    with tile.TileContext(nc) as tc:
        with (
            tc.tile_pool(name="persist", bufs=1) as pp,
            tc.tile_pool(name="rot", bufs=3) as rot,
            tc.tile_pool(name="rot2", bufs=2) as rot2,
            tc.tile_pool(name="psum", bufs=2, space="PSUM") as ps,
        ):
            # ---- persistent SBUF tiles ----
            xwa = pp.tile([128, NB, W], bf16, tag="xwa")
            wqa = pp.tile([128, NB, DIM], bf16, tag="wqa")
            wka = pp.tile([128, NB, DIM], bf16, tag="wka")
            wva = pp.tile([128, NB, DIM], bf16, tag="wva")
            woa = pp.tile([128, NB, DIM], bf16, tag="woa")
            xw = [xwa[:, i, :] for i in range(NB)]
            wq = [wqa[:, i, :] for i in range(NB)]
            wk = [wka[:, i, :] for i in range(NB)]
            wv = [wva[:, i, :] for i in range(NB)]
            wo = [woa[:, i, :] for i in range(NB)]
            qt = [pp.tile([128, CH], bf16, tag=f"qt{i}", name=f"qt{i}") for i in range(NB)]
            # per-head K, feature rows zero-padded to 128, key cols zero-padded
            # to WPAD so every score matmul is a full [128,128] lhsT
            kt2 = [pp.tile([128, WPAD], bf16, tag=f"kt{i}", name=f"kt{i}") for i in range(H)]
            # V: tokens on partitions; per head 64 value cols + 1 ones col
            vt = [pp.tile([128, H, D + 1], bf16, tag=f"vt{i}", name=f"vt{i}") for i in range(5)]
            band = pp.tile([128, 512], bf16, tag="band")
            bo_sb = pp.tile([1, DIM], bf16, tag="bo")
            bo_bc = pp.tile([128, DIM], fp32, tag="bo_bc")
            ones = pp.tile([1, 128], bf16, tag="ones")
            ident = pp.tile([128, 128], bf16, tag="ident")
            scr = pp.tile([128, 256], bf16, tag="scr")

            # ---- PE warm-up (HAM 8/8 before the first real matmul) ----
            nc.vector.memset(scr[:], 0.0)
            nc.vector.memset(ones[:], 1.0)
            for _ in range(12):
                wps = ps.tile([128, 256], fp32, tag="proj", name="warm_ps", bufs=4)
                nc.tensor.matmul(wps[:], scr[:, 0:128], scr[:],
                                 start=True, stop=True)

            # ---- input DMAs, ordered by need time ----
            # scalar HW queue: ONLY xw (8 issues, drained early) so the Q/K
            # projection PSUM->SBUF copies on the scalar engine are never
            # blocked behind DMA ring-credit waits.
            for i in range(NB):
                nc.scalar.dma_start(out=xwa[:, i, :], in_=xw_d[i * 128:(i + 1) * 128, :])
            # sync HW queue: wq (needed from ~8us) then wk (needed from ~22us)
            # then the small mask/bias tensors. (Splitting wq into halves was
            # tried: the extra issue instructions delay the wk issues enough
            # to stall the K projection.)
            for i in range(NB):
                nc.sync.dma_start(out=wqa[:, i, :], in_=wq_d[i * 128:(i + 1) * 128, :])
            for i in range(NB):
                nc.sync.dma_start(out=wka[:, i, :], in_=wk_d[i * 128:(i + 1) * 128, :])
            nc.sync.dma_start(out=band[:], in_=band_d[:])
            nc.sync.dma_start(out=bo_sb[:], in_=bo_d[:])
            # gpsimd software queue: pad memsets first (delays wv/wo issue so
            # early DMA bandwidth goes to xw/wq/wk), then wv, then wo.
            make_identity(nc, ident[:])
            # vt: ones-column (col D survives the V copies) + vt[4] tail rows
            # (tokens 575.. are never written; AV reads them against masked
            # zeros, so any finite value works).
            for t in range(5):
                nc.gpsimd.memset(vt[t][:, :, D], 1.0)
            # (partition starts must be 32-aligned with limited spans; rows
            # 32..62 get overwritten by the V copies afterwards)
            nc.gpsimd.memset(vt[4][32:64, :, :], 1.0)
            nc.gpsimd.memset(vt[4][64:128, :, :], 1.0)
            # kt2: zero the other-head feature rows + the key-column pad
            for h in range(H):
                lo, hi = (64, 128) if h % 2 == 0 else (0, 64)
                dlo, dhi = (0, 64) if h % 2 == 0 else (64, 128)
                nc.gpsimd.memset(kt2[h][lo:hi, :], 0.0)
                nc.gpsimd.memset(kt2[h][dlo:dhi, W:WPAD], 0.0)
            for i in range(NB):
                nc.gpsimd.dma_start(out=wva[:, i, :], in_=wv_d[i * 128:(i + 1) * 128, :])
            for i in range(NB):
                nc.gpsimd.dma_start(out=woa[:, i, :], in_=wo_d[i * 128:(i + 1) * 128, :])

            # ---- Q projection (1/sqrt(D) pre-folded into wq on the host) ----
            # d-outer / e-inner over 4 concurrent PSUM banks: each arriving
            # (wq, xw) chunk pair unlocks 4 back-to-back matmuls.
            for half in range(2):
                accs = [ps.tile([128, CH], fp32, tag="proj", name="proj_ps",
                                bufs=4) for _ in range(4)]
                for d in range(NB):
                    for e4 in range(4):
                        e = half * 4 + e4
                        nc.tensor.matmul(accs[e4][:],
                                         wq[d][:, e * 128:(e + 1) * 128],
                                         xw[d][:, HALF:HALF + CH],
                                         start=(d == 0), stop=(d == NB - 1))
                for e4 in range(4):
                    if e4 % 2 == 0:
                        nc.vector.tensor_copy(qt[half * 4 + e4][:], accs[e4][:])
                    else:
                        nc.scalar.copy(qt[half * 4 + e4][:], accs[e4][:])

            # ---- K projection into zero-padded per-head tiles ----
            # d-outer over 2 concurrent banks so the wk stream is consumed as
            # it lands
            for t0, tn in ((0, 288), (288, W - 288)):
                for quarter in range(4):
                    accs = [ps.tile([128, CH], fp32, tag="proj", name="proj_ps",
                                    bufs=4) for _ in range(2)]
                    for d in range(NB):
                        for e2 in range(2):
                            e = quarter * 2 + e2
                            nc.tensor.matmul(accs[e2][:, 0:tn],
                                             wk[d][:, e * 128:(e + 1) * 128],
                                             xw[d][:, t0:t0 + tn],
                                             start=(d == 0), stop=(d == NB - 1))
                    for e2 in range(2):
                        e = quarter * 2 + e2
                        if e2 % 2 == 0:
                            nc.scalar.copy(kt2[2 * e][0:64, t0:t0 + tn],
                                           accs[e2][0:64, 0:tn])
                            nc.scalar.copy(kt2[2 * e + 1][64:128, t0:t0 + tn],
                                           accs[e2][64:128, 0:tn])
                        else:
                            nc.vector.tensor_copy(kt2[2 * e][0:64, t0:t0 + tn],
                                                  accs[e2][0:64, 0:tn])
                            nc.vector.tensor_copy(kt2[2 * e + 1][64:128, t0:t0 + tn],
                                                  accs[e2][64:128, 0:tn])

            # ---- broadcast output bias to all 128 partitions (K=1 matmul) ----
            for eh in range(2):
                bps = ps.tile([128, 512], fp32, tag="sc", name="bo_ps", bufs=2)
                nc.tensor.matmul(bps[:], ones[:, 0:128],
                                 bo_sb[:, eh * 512:(eh + 1) * 512],
                                 start=True, stop=True)
                nc.vector.tensor_copy(bo_bc[:, eh * 512:(eh + 1) * 512], bps[:])

            # ---- V projection: vt[t][tok, h, 0:64]; col 64 stays 1.0 ----
            vgroups = [(t, eh) for t in range(5) for eh in range(2)]
            for pg in range(0, 10, 2):
                gs = vgroups[pg:pg + 2]
                accs = [ps.tile([128, 8, D], fp32, tag="proj", name="proj_ps",
                                bufs=4) for _ in gs]
                for d in range(NB):
                    for gi, (t, eh) in enumerate(gs):
                        npart = 128 if t < 4 else W - 512    # 63 in last chunk
                        nc.tensor.matmul(accs[gi][0:npart],
                                         xw[d][:, t * 128:t * 128 + npart],
                                         wv[d][:, eh * 512:(eh + 1) * 512],
                                         start=(d == 0), stop=(d == NB - 1))
                for gi, (t, eh) in enumerate(gs):
                    npart = 128 if t < 4 else W - 512
                    if gi % 2 == 0:
                        nc.vector.tensor_copy(vt[t][0:npart, eh * 8:(eh + 1) * 8, 0:D],
                                              accs[gi][0:npart])
                    else:
                        nc.scalar.copy(vt[t][0:npart, eh * 8:(eh + 1) * 8, 0:D],
                                       accs[gi][0:npart])

            # ---- attention + output projection, per 128-query block ----
            for qb in range(QB):
                q0 = qb * 128
                # attnout as [128, chunk, head-in-pair, 64] so the pair
                # normalization is a single 3D tensor_tensor per pair
                attnout = rot2.tile([128, NB, 2, D], bf16, tag="attnout",
                                    name="attnout")

                def scores(p):
                    """Scores+band-bias+exp for head pair (2p, 2p+1), one bank.

                    The -30000 window bias is accumulated into PSUM by a fifth
                    matmul (identity @ band), so exp underflows to exact zeros
                    out-of-band and no separate mask op is needed. (A separate
                    mask multiply was tried on GpSimd/DVE; no helper engine has
                    both the throughput and the slack, so the PE matmul is the
                    cheapest masked-add.)"""
                    sc = ps.tile([128, 512], fp32, tag="sc", name="sc_ps", bufs=2)
                    for hh in range(2):
                        h = 2 * p + hh
                        nc.tensor.matmul(sc[:, 256 * hh:256 * hh + 128],
                                         kt2[h][:, q0:q0 + 128],
                                         qt[p][:, q0:q0 + 128],
                                         start=(hh == 0), stop=False)
                        nc.tensor.matmul(sc[:, 256 * hh + 128:256 * hh + 256],
                                         kt2[h][:, q0 + 128:q0 + 256],
                                         qt[p][:, q0:q0 + 128],
                                         start=False, stop=False)
                    nc.tensor.matmul(sc[:], ident[:], band[:],
                                     start=False, stop=True)
                    e_sb = rot.tile([128, 512], bf16, tag="e", name="e_sb", bufs=4)
                    nc.scalar.activation(e_sb[:], sc[:], Exp)
                    return e_sb

                def av_pair(p, e_sb):
                    av = ps.tile([128, 2, D + 1], fp32, tag="av", name="av_ps", bufs=2)
                    for hh in range(2):
                        nc.tensor.matmul(av[:, hh, :], e_sb[:, 256 * hh:256 * hh + 128],
                                         vt[qb][:, 2 * p + hh, :],
                                         start=(hh == 0), stop=False)
                        nc.tensor.matmul(av[:, hh, :], e_sb[:, 256 * hh + 128:256 * hh + 256],
                                         vt[qb + 1][:, 2 * p + hh, :],
                                         start=False, stop=(hh == 1))
                    invden = rot.tile([128, 2, 1], fp32, tag="invden", name="invden",
                                      bufs=4)
                    nc.vector.reciprocal(invden[:, :, 0], av[:, :, D])
                    inv_b = invden[:, :, :].broadcast_to([128, 2, D])
                    nc.vector.tensor_mul(attnout[:, p, :, :], av[:, :, 0:D],
                                         inv_b)

                # transpose attnout chunk p to [dim, q] right after its
                # normalization lands. NOT transpose-mode (387ns measured -
                # it pays PE_SBUF_ACCESS_LATENCY and skips HAM credit):
                # a regular matmul with the chunk as the STATIONARY operand
                # and identity as the moving one computes chunk.T @ I =
                # chunk^T at plain-matmul cost (~60ns). (The DMA XBAR
                # alternative costs 1.24us of issuing-engine time per 32KB.)
                attnT = []

                def transpose_chunk(c):
                    trp = ps.tile([128, 128], bf16, tag="sc", name="tr_ps", bufs=2)
                    nc.tensor.transpose(trp[:], attnout[:, c, :, :], ident[:])
                    at = rot2.tile([128, 128], bf16, tag=f"attnT{c}", name=f"attnT{c}")
                    nc.vector.tensor_copy(at[:], trp[:])
                    attnT.append(at)

                es = [scores(0), scores(1)]
                for p in range(8):
                    if p + 2 < 8:
                        es.append(scores(p + 2))
                    av_pair(p, es[p])
                    if p >= 1:
                        transpose_chunk(p - 1)

                # O projection; bias added via the broadcast tile on DVE.
                # Both eh accumulations for chunks 0..6 are issued before the
                # last transpose so the PE has work while chunk 7's
                # normalization drains.
                out_sb = rot2.tile([128, DIM], bf16, tag="out", name="out_sb")
                acc0 = ps.tile([128, 512], fp32, tag="proj", name="proj_ps", bufs=4)
                acc1 = ps.tile([128, 512], fp32, tag="proj", name="proj_ps", bufs=4)
                for c in range(7):
                    nc.tensor.matmul(acc0[:], attnT[c][:], wo[c][:, 0:512],
                                     start=(c == 0), stop=False)
                for c in range(7):
                    nc.tensor.matmul(acc1[:], attnT[c][:], wo[c][:, 512:1024],
                                     start=(c == 0), stop=False)
                transpose_chunk(7)
                nc.tensor.matmul(acc0[:], attnT[7][:], wo[7][:, 0:512],
                                 start=False, stop=True)
                last = qb == QB - 1
                nc.tensor.matmul(acc1[:], attnT[7][:], wo[7][:, 512:1024],
                                 start=False, stop=not last)
                if last:
                    # final drain: fold the eh=1 bias into PSUM via a K=1
                    # ones-matmul so the two drains (DVE add / scalar copy)
                    # and the three stores all run in parallel.
                    nc.tensor.matmul(acc1[:], ones[:, 0:128],
                                     bo_sb[:, 512:1024],
                                     start=False, stop=True)
                    nc.vector.tensor_add(out_sb[:, 0:512], acc0[:],
                                         bo_bc[:, 0:512])
                    nc.scalar.copy(out_sb[:, 512:1024], acc1[:])
                    nc.sync.dma_start(out=out_d[0, q0:q0 + 128, :],
                                      in_=out_sb[:, 0:512])
                    nc.scalar.dma_start(out=out_d[1, q0:q0 + 64, :],
                                        in_=out_sb[0:64, 512:1024])
                    nc.gpsimd.dma_start(out=out_d[1, q0 + 64:q0 + 128, :],
                                        in_=out_sb[64:128, 512:1024])
                else:
                    # mid-stream: bias adds on DVE, stores on the idle gpsimd
                    # queue (sync is busy issuing the XBAR transposes)
                    for eh, acc in ((0, acc0), (1, acc1)):
                        nc.vector.tensor_add(out_sb[:, eh * 512:(eh + 1) * 512],
                                             acc[:],
                                             bo_bc[:, eh * 512:(eh + 1) * 512])
                        nc.gpsimd.dma_start(
                            out=out_d[eh, q0:q0 + 128, :],
                            in_=out_sb[:, eh * 512:(eh + 1) * 512])

    nc.compile()
    return nc


def _host_prep(x, Wq, Wk, Wv, Wo, bo):
    """Per-core input maps: transposed bf16 weights + clamp-gathered x^T windows."""
    wqT = np.ascontiguousarray(Wq.T * SCALE).astype(BF16)   # fold 1/sqrt(D)
    wkT = np.ascontiguousarray(Wk.T).astype(BF16)
    wvT = np.ascontiguousarray(Wv.T).astype(BF16)
    woT = np.ascontiguousarray(Wo.T).astype(BF16)
    bo2 = bo.reshape(1, DIM).astype(BF16)

    # additive band bias, [key, query] layout, repeated for a head pair:
    # cols [A | B | A | B]; 0 in-band, -30000 out-of-band (exp underflows to 0)
    r = np.arange(128)[:, None]
    qq = np.arange(128)[None, :]
    bandA = np.where((r - qq >= 0) & (r - qq <= 63), 0.0, -30000.0)
    bandB = np.where((128 + r - qq >= 0) & (128 + r - qq <= 63), 0.0, -30000.0)
    band = np.concatenate([bandA, bandB, bandA, bandB], axis=1).astype(BF16)

    in_maps = []
    for core in range(N_CORES):
        b, c = divmod(core, QB)
        c0 = c * CH
        idx = np.clip(np.arange(c0 - HALF, c0 + CH + HALF - 1), 0, S - 1)
        xwin = np.ascontiguousarray(x[b].T[:, idx]).astype(BF16)
        in_maps.append({
            "xw": xwin, "wq": wqT, "wk": wkT, "wv": wvT, "wo": woT,
            "bo": bo2, "band": band,
        })
    return in_maps


def _run(x, Wq, Wk, Wv, Wo, bo, trace=False, **kw):
    if "nc" not in _CACHED:
        _CACHED["nc"] = _build_nc()
    nc = _CACHED["nc"]
    in_maps = _host_prep(x, Wq, Wk, Wv, Wo, bo)
    res = run_bass_kernel_spmd(nc, in_maps, list(range(N_CORES)),
                               trace=trace, **kw)
    out = np.empty((B, S, DIM), np.float32)
    for core in range(N_CORES):
        b, c = divmod(core, QB)
        o = res.results[core]["out"].astype(np.float32)   # [2, CH, 512]
        out[b, c * CH:(c + 1) * CH, 0:512] = o[0]
        out[b, c * CH:(c + 1) * CH, 512:1024] = o[1]
    return out, res


def kernel(x, cantor_coords, Wq, Wk, Wv, Wo, bo):
    x = np.asarray(x, dtype=np.float32)
    out, _ = _run(x, np.asarray(Wq), np.asarray(Wk), np.asarray(Wv),
                  np.asarray(Wo), np.asarray(bo))
    return out
